# revision 34
# baseline (speedup 1.0000x reference)
"""CQAttention Trainium2 kernel.

Math (per batch b, H=256, q=2048, d=8192):
  Qp   = gelu(Q @ W.T + b)                       [q, H]
  S    = C @ Qp.T                                [d, q]
  P    = softmax(S, axis=q)
  out  = P @ Qp + C                              [d, H]

Sharding: data-parallel over batch, one batch per NeuronCore (8 cores).

Per-core pipeline:
  - Q and C load with a contiguous per-partition DRAM mapping "(p a) h"
    (one 4KB descriptor per partition instead of 4-16 strided 1KB ones);
    softmax over q is permutation-invariant and the d-mapping only needs
    to be consistent between ct / c_nat / the output store, so the
    row-scramble is free. Loads are f32 HWDGE DMAs split across the two
    DGE sequencers (ACT idle at startup, SP).
  - Startup transposes (W^T, all Q^T groups, C^T chunk 0) run on the PE
    straight from f32 (2 cyc/row; the psum->SBUF copy casts to fp16, so
    no separate cast stage exists on any startup chain). C^T for chunks
    >= 1 runs on the DMA XBAR (dma_start_transpose, 14ns/16x128 tile)
    from a fp16 copy made by the otherwise-idle Pool engine, keeping the
    PE free for matmuls. ~22 dummy matmuls at the very start hold the PE
    busy through the first DMA waits so the p-state ramp (full clock
    after ~3us continuous execution) completes before real work.
  - QpT = gelu(W Q^T + b) with per-partition bias on ACT; Qp (natural,
    bf16) by PE-transposing QpT back, with a ones column so the softmax
    denominator falls out of the attended matmul's PSUM accumulation.
  - Per 512-row chunk of C: logits^T tiles [q=128, d=512] with fp16
    operands (bf16 fails the 2e-2 gate); exp on ACT straight from PSUM
    to bf16 (no max-subtraction: |logits| < ~70 so fp32 exp is safe);
    attended accumulated over 16 q-tiles into PSUM [d=128, 257] whose
    column 256 is the row-sum; fused epilogue
    out = (attended * 1/rowsum) + C in one DVE op per tile.
  - Chunk pipeline: C loads 3 chunks ahead, XBAR transposes 2 ahead,
    attended lags logits/exp by 2 q-tiles.
"""

from contextlib import ExitStack

import numpy as np

import concourse.mybir as mybir
import concourse.tile as tile
from concourse import bacc
from concourse.bass_utils import run_bass_kernel_spmd
from concourse.masks import make_identity

B, QL, D, H = 8, 2048, 8192, 256
N_CORES = 8
F32 = mybir.dt.float32
BF16 = mybir.dt.bfloat16
F16 = mybir.dt.float16

HC = H // 128      # feature chunks (2)
NQT = QL // 128    # q tiles (16)
NQG = NQT // 4     # q groups of 4 tiles (4)
DC = 512           # d-chunk size
NDC = D // DC      # d chunks (16)
NDM = DC // 128    # d tiles per chunk (4)

LS = F16  # logits-matmul operand dtype

AF = mybir.ActivationFunctionType
ALU = mybir.AluOpType


def build_body(ctx: ExitStack, tc: tile.TileContext, nc, Qd, Cd, Wd, bd, Od):
    singles = ctx.enter_context(tc.tile_pool(name="singles", bufs=1))
    qstat = ctx.enter_context(tc.tile_pool(name="qstat", bufs=1))
    cpool = ctx.enter_context(tc.tile_pool(name="cpool", bufs=5))
    ctpool = ctx.enter_context(tc.tile_pool(name="ctp", bufs=4))
    exppool = ctx.enter_context(tc.tile_pool(name="expp", bufs=2))
    outpool = ctx.enter_context(tc.tile_pool(name="outp", bufs=3))
    small = ctx.enter_context(tc.tile_pool(name="small", bufs=4))
    psum_l = ctx.enter_context(tc.tile_pool(name="psl", bufs=2, space="PSUM"))
    psum_t = ctx.enter_context(tc.tile_pool(name="pst", bufs=2, space="PSUM"))
    psum_a = ctx.enter_context(tc.tile_pool(name="psa", bufs=1, space="PSUM"))

    ident = singles.tile([128, 128], LS)
    make_identity(nc, ident)

    # Dummy matmuls to ramp the PE out of its p-state throttle (full clock
    # needs ~3us of continuous execution) while the first loads are in
    # flight; results are never read.
    warm = singles.tile([128, 512], LS, name="warm")
    nc.gpsimd.memset(warm[:], 0.0)
    warm_o = singles.tile([128, 1], F32, name="warm_o")
    nc.scalar.activation(warm_o[:], warm[:, 0:1], AF.Gelu)
    def warmup(n):
        for _ in range(n):
            pw = psum_l.tile([128, 512], F32, tag="pl", name="warmup")
            nc.tensor.matmul(pw[:], ident[:], warm[:], start=True, stop=True)

    warmup(22)

    # --- loads alternate between the two HWDGE queues (ACT idle at start,
    # SP) so the four Q transfers overlap ---
    ident32 = singles.tile([128, 128], F32)
    make_identity(nc, ident32)

    q_nat = cpool.tile([128, NQT, H], F32, tag="qnat", bufs=1)
    q_view = Qd.rearrange("(p a) h -> p a h", p=128)
    nc.scalar.dma_start(out=q_nat[:, 0:4, :], in_=q_view[:, 0:4, :])
    w_nat = singles.tile([128, HC, H], F32)  # [o in-chunk, om, h]
    nc.scalar.dma_start(out=w_nat[:],
                        in_=Wd.rearrange("(a p) h -> p a h", p=128))

    c_nats = {}
    c_bfs = {}

    def c_load(dc, make_bf=True):
        c_nats[dc] = cpool.tile([128, NDM, H], F32, tag="cnat", name=f"cnat{dc}")
        nc.sync.dma_start(
            out=c_nats[dc][:],
            in_=Cd[dc * DC:(dc + 1) * DC, :].rearrange("(p a) h -> p a h", p=128))
        if make_bf:
            c_bfs[dc] = cpool.tile([128, NDM, H], LS, tag="cbf", name=f"cbf{dc}")
            nc.gpsimd.tensor_copy(c_bfs[dc][:], c_nats[dc][:])

    c_load(0, make_bf=False)
    nc.sync.dma_start(out=q_nat[:, 4:8, :], in_=q_view[:, 4:8, :])
    bias = singles.tile([128, HC, 1], F32)
    nc.scalar.dma_start(out=bias[:, :, 0], in_=bd.rearrange("(c p) -> p c", p=128))
    nc.scalar.dma_start(out=q_nat[:, 8:12, :], in_=q_view[:, 8:12, :])
    nc.sync.dma_start(out=q_nat[:, 12:16, :], in_=q_view[:, 12:16, :])
    c_load(1)
    c_load(2)
    c_load(3)

    # --- C^T: ct[h, dm, hc, d] = C^{(pa)}[dc*512, dm, hc, d] ---
    cts = {}

    def c_prep(dc, on_pe=False):
        cts[dc] = ctpool.tile([128, NDM, HC, 128], LS, tag="ct", name=f"ct{dc}")
        if on_pe:
            for hc in range(HC):
                pt = psum_t.tile([128, 512], F32, tag="pt", name=f"ptc{dc}_{hc}")
                for dm in range(NDM):
                    nc.tensor.transpose(
                        pt[:, dm * 128:(dm + 1) * 128],
                        c_nats[dc][:, dm, hc * 128:(hc + 1) * 128], ident32[:])
                nc.vector.tensor_copy(
                    cts[dc][:, :, hc, :], pt.rearrange("p (a b) -> p a b", a=4))
        else:
            nc.sync.dma_start_transpose(cts[dc][:], c_bfs[dc][:])

    # qt[h, g, k, hc, q] = Q[g*512 + k*128 + q, hc*128 + h]
    # All transposes straight from f32 on the PE (2 cyc/row); the
    # psum->SBUF copy casts to fp16, so no separate cast stage exists on
    # any startup chain.
    qt = qstat.tile([128, NQG, 4, HC, 128], LS)

    def qt_pe(g):
        for hc in range(HC):
            pt = psum_t.tile([128, 512], F32, tag="pt", name=f"ptq{g}_{hc}")
            for k in range(4):
                nc.tensor.transpose(
                    pt[:, k * 128:(k + 1) * 128],
                    q_nat[:, g * 4 + k, hc * 128:(hc + 1) * 128], ident32[:])
            nc.vector.tensor_copy(
                qt[:, g, :, hc, :], pt.rearrange("p (a b) -> p a b", a=4))

    qt_pe(0)

    # --- W^T on PE: wt[h, hc, om*128+o] = W[om*128+o, hc*128+h] ---
    wt = qstat.tile([128, HC, H], LS)
    for om in range(HC):
        for hc in range(HC):
            pt = psum_t.tile([128, 128], F32, tag="pt", name=f"ptw{om}_{hc}")
            nc.tensor.transpose(pt[:], w_nat[:, om, hc * 128:(hc + 1) * 128],
                                ident32[:])
            nc.vector.tensor_copy(wt[:, hc, om * 128:(om + 1) * 128], pt[:])

    qt_pe(1)
    qt_pe(2)
    qt_pe(3)

    c_prep(0, on_pe=True)

    # --- per-group pipeline: linear+gelu -> QpT -> Qp ---
    qpt = qstat.tile([128, HC, QL], LS)
    qp = qstat.tile([128, NQT, H + 1], BF16)

    def lin_part(qg):
        # linear + gelu for this 512-wide q block
        for om in range(HC):
            pl = psum_l.tile([128, 512], F32, tag="pl", name=f"plin{qg}_{om}")
            for hc in range(HC):
                nc.tensor.matmul(
                    pl[:],
                    wt[:, hc, om * 128:(om + 1) * 128],
                    qt[:, qg, :, hc, :],
                    start=(hc == 0),
                    stop=(hc == HC - 1),
                )
            nc.scalar.activation(
                qpt[:, om, qg * 512:(qg + 1) * 512], pl[:], AF.Gelu,
                bias=bias[:, om, :], scale=1.0,
            )

    def qpb_part(qg):
        # Qp natural for this group (PE back-transpose)
        for om in range(HC):
            pt = psum_t.tile([128, 512], LS, tag="pt", name=f"ptp{qg}_{om}")
            for k in range(4):
                qi = qg * 4 + k
                nc.tensor.transpose(
                    pt[:, k * 128:(k + 1) * 128],
                    qpt[:, om, qi * 128:(qi + 1) * 128], ident[:])
            nc.vector.tensor_copy(
                qp[:, qg * 4:(qg + 1) * 4, om * 128:(om + 1) * 128],
                pt.rearrange("p (a b) -> p a b", a=4))

    lin_part(0)
    qpb_part(0)
    c_prep(1)
    lin_part(1)
    qpb_part(1)
    lin_part(2)
    qpb_part(2)
    lin_part(3)
    qpb_part(3)
    nc.vector.memset(qp[:, :, H:H + 1], 1.0)

    # Lag the attended matmuls behind logits+exp so the PE never waits on
    # the ACT exp latency. Chunk 0 uses a deeper lag: its first exp sits
    # behind gelu-g3 plus the exp-table load on ACT, so the logits run
    # ahead through extra PSUM slots instead of head-of-line blocking.
    LAG = 2
    LAG0 = 2
    for dc in range(NDC):
        lag = LAG0 if dc == 0 else LAG
        c_nat = c_nats[dc]
        ct = cts[dc]
        expt = exppool.tile([128, NQT, DC], BF16, tag="expt")
        pa = [psum_a.tile([128, H + 1], F32, tag=f"a{dm}", name=f"pa{dm}")
              for dm in range(NDM)]
        for step in range(NQT + lag):
            if step == 2 and dc >= 1 and dc + 3 < NDC:
                c_load(dc + 3)
            if step == 6 and dc + 2 < NDC and dc + 2 > 1:
                c_prep(dc + 2)
            if step < NQT:
                qi = step
                if qi in (5, 13) and dc > 0:
                    pl = psum_t.tile([128, DC], F32, tag="pt",
                                     name=f"plx{dc}_{qi}")
                else:
                    pl = psum_l.tile([128, DC], F32, tag="pl")
                for hc in range(HC):
                    nc.tensor.matmul(
                        pl[:],
                        qpt[:, hc, qi * 128:(qi + 1) * 128],
                        ct[:, :, hc, :],
                        start=(hc == 0),
                        stop=(hc == HC - 1),
                    )
                nc.scalar.activation(expt[:, qi, :], pl[:], AF.Exp)
            if step >= lag:
                qj = step - lag
                for dm in range(NDM):
                    nc.tensor.matmul(
                        pa[dm][:],
                        expt[:, qj, dm * 128:(dm + 1) * 128],
                        qp[:, qj, :],
                        start=(qj == 0),
                        stop=(qj == NQT - 1),
                    )

        o_sb = outpool.tile([128, NDM, H], F32)
        if dc == NDC - 1:
            halves = ((0, 1), (1, 2), (2, 3), (3, 4))
        else:
            halves = ((0, 4),)
        for lo, hi in halves:
            for dm in range(lo, hi):
                rec = small.tile([128, 1], F32)
                nc.vector.reciprocal(rec[:], pa[dm][:, H:H + 1])
                nc.vector.scalar_tensor_tensor(
                    o_sb[:, dm, :], pa[dm][:, 0:H], rec[:], c_nat[:, dm, :],
                    ALU.mult, ALU.add,
                )
            nc.sync.dma_start(
                out=Od[dc * DC:(dc + 1) * DC, :]
                .rearrange("(p a) h -> p a h", p=128)[:, lo:hi, :],
                in_=o_sb[:, lo:hi, :])
        del c_nats[dc], cts[dc]
        c_bfs.pop(dc, None)


def build_nc():
    nc = bacc.Bacc("TRN2", target_bir_lowering=False, debug=False,
                   num_devices=N_CORES)
    Qd = nc.dram_tensor("Q", [QL, H], F32, kind="ExternalInput")
    Cd = nc.dram_tensor("C", [D, H], F32, kind="ExternalInput")
    Wd = nc.dram_tensor("W", [H, H], F32, kind="ExternalInput")
    bd = nc.dram_tensor("b", [H], F32, kind="ExternalInput")
    Od = nc.dram_tensor("out", [D, H], F32, kind="ExternalOutput")
    with tile.TileContext(nc) as tc:
        with ExitStack() as ctx:
            build_body(ctx, tc, nc, Qd[:], Cd[:], Wd[:], bd[:], Od[:])
    nc.finalize()
    return nc


_NC = None


def get_nc():
    global _NC
    if _NC is None:
        _NC = build_nc()
    return _NC


def kernel(Q, C, W, b):
    assert Q.shape == (B, QL, H) and C.shape == (B, D, H)
    nc = get_nc()
    in_maps = [
        {
            "Q": np.ascontiguousarray(Q[i], dtype=np.float32),
            "C": np.ascontiguousarray(C[i], dtype=np.float32),
            "W": np.ascontiguousarray(W, dtype=np.float32),
            "b": np.ascontiguousarray(b, dtype=np.float32),
        }
        for i in range(N_CORES)
    ]
    res = run_bass_kernel_spmd(nc, in_maps, core_ids=list(range(N_CORES)))
    return np.stack([res.results[i]["out"] for i in range(N_CORES)], axis=0)


# revision 35
# speedup vs baseline: 1.0129x; 1.0129x over previous
"""CQAttention Trainium2 kernel.

Math (per batch b, H=256, q=2048, d=8192):
  Qp   = gelu(Q @ W.T + b)                       [q, H]
  S    = C @ Qp.T                                [d, q]
  P    = softmax(S, axis=q)
  out  = P @ Qp + C                              [d, H]

Sharding: data-parallel over batch, one batch per NeuronCore (8 cores).

Per-core pipeline:
  - Q and C load with a contiguous per-partition DRAM mapping "(p a) h"
    (one 4KB descriptor per partition instead of 4-16 strided 1KB ones);
    softmax over q is permutation-invariant and the d-mapping only needs
    to be consistent between ct / c_nat / the output store, so the
    row-scramble is free. Loads are f32 HWDGE DMAs split across the two
    DGE sequencers (ACT idle at startup, SP).
  - Startup transposes (W^T, all Q^T groups, C^T chunk 0) run on the PE
    straight from f32 (2 cyc/row; the psum->SBUF copy casts to fp16, so
    no separate cast stage exists on any startup chain). C^T for chunks
    >= 1 runs on the DMA XBAR (dma_start_transpose, 14ns/16x128 tile)
    from a fp16 copy made by the otherwise-idle Pool engine, keeping the
    PE free for matmuls. ~22 dummy matmuls at the very start hold the PE
    busy through the first DMA waits so the p-state ramp (full clock
    after ~3us continuous execution) completes before real work.
  - QpT = gelu(W Q^T + b) with per-partition bias on ACT; Qp (natural,
    bf16) by PE-transposing QpT back, with a ones column so the softmax
    denominator falls out of the attended matmul's PSUM accumulation.
  - Per 512-row chunk of C: logits^T tiles [q=128, d=512] with fp16
    operands (bf16 fails the 2e-2 gate); exp on ACT straight from PSUM
    to bf16 (no max-subtraction: |logits| < ~70 so fp32 exp is safe);
    attended accumulated over 16 q-tiles into PSUM [d=128, 257] whose
    column 256 is the row-sum; fused epilogue
    out = (attended * 1/rowsum) + C in one DVE op per tile.
  - Chunk pipeline: C loads 3 chunks ahead, XBAR transposes 2 ahead,
    attended lags logits/exp by 2 q-tiles.
"""

from contextlib import ExitStack

import numpy as np

import concourse.mybir as mybir
import concourse.tile as tile
from concourse import bacc
from concourse.bass_utils import run_bass_kernel_spmd
from concourse.masks import make_identity

B, QL, D, H = 8, 2048, 8192, 256
N_CORES = 8
F32 = mybir.dt.float32
BF16 = mybir.dt.bfloat16
F16 = mybir.dt.float16

HC = H // 128      # feature chunks (2)
NQT = QL // 128    # q tiles (16)
NQG = NQT // 4     # q groups of 4 tiles (4)
DC = 512           # d-chunk size
NDC = D // DC      # d chunks (16)
NDM = DC // 128    # d tiles per chunk (4)

LS = F16  # logits-matmul operand dtype

AF = mybir.ActivationFunctionType
ALU = mybir.AluOpType


def build_body(ctx: ExitStack, tc: tile.TileContext, nc, Qd, Cd, Wd, bd, Od):
    singles = ctx.enter_context(tc.tile_pool(name="singles", bufs=1))
    qstat = ctx.enter_context(tc.tile_pool(name="qstat", bufs=1))
    cpool = ctx.enter_context(tc.tile_pool(name="cpool", bufs=5))
    ctpool = ctx.enter_context(tc.tile_pool(name="ctp", bufs=4))
    exppool = ctx.enter_context(tc.tile_pool(name="expp", bufs=2))
    outpool = ctx.enter_context(tc.tile_pool(name="outp", bufs=3))
    small = ctx.enter_context(tc.tile_pool(name="small", bufs=4))
    psum_l = ctx.enter_context(tc.tile_pool(name="psl", bufs=2, space="PSUM"))
    psum_t = ctx.enter_context(tc.tile_pool(name="pst", bufs=2, space="PSUM"))
    psum_a = ctx.enter_context(tc.tile_pool(name="psa", bufs=1, space="PSUM"))

    ident = singles.tile([128, 128], LS)
    make_identity(nc, ident)

    # Dummy matmuls to ramp the PE out of its p-state throttle (full clock
    # needs ~3us of continuous execution) while the first loads are in
    # flight; results are never read.
    warm = singles.tile([128, 512], LS, name="warm")
    nc.gpsimd.memset(warm[:], 0.0)
    warm_o = singles.tile([128, 1], F32, name="warm_o")
    nc.scalar.activation(warm_o[:], warm[:, 0:1], AF.Gelu)
    def warmup(n):
        for _ in range(n):
            pw = psum_l.tile([128, 512], F32, tag="pl", name="warmup")
            nc.tensor.matmul(pw[:], ident[:], warm[:], start=True, stop=True)

    warmup(22)

    # --- loads alternate between the two HWDGE queues (ACT idle at start,
    # SP) so the four Q transfers overlap ---
    ident32 = singles.tile([128, 128], F32)
    make_identity(nc, ident32)

    q_nat = cpool.tile([128, NQT, H], F32, tag="qnat", bufs=1)
    q_view = Qd.rearrange("(p a) h -> p a h", p=128)
    nc.scalar.dma_start(out=q_nat[:, 0:4, :], in_=q_view[:, 0:4, :])
    w_nat = singles.tile([128, HC, H], F32)  # [o in-chunk, om, h]
    nc.scalar.dma_start(out=w_nat[:],
                        in_=Wd.rearrange("(a p) h -> p a h", p=128))

    c_nats = {}
    c_bfs = {}

    def c_load(dc, make_bf=True):
        c_nats[dc] = cpool.tile([128, NDM, H], F32, tag="cnat", name=f"cnat{dc}")
        nc.sync.dma_start(
            out=c_nats[dc][:],
            in_=Cd[dc * DC:(dc + 1) * DC, :].rearrange("(p a) h -> p a h", p=128))
        if make_bf:
            c_bfs[dc] = cpool.tile([128, NDM, H], LS, tag="cbf", name=f"cbf{dc}")
            nc.gpsimd.tensor_copy(c_bfs[dc][:], c_nats[dc][:])

    c_load(0, make_bf=False)
    nc.sync.dma_start(out=q_nat[:, 4:8, :], in_=q_view[:, 4:8, :])
    bias = singles.tile([128, HC, 1], F32)
    nc.scalar.dma_start(out=bias[:, :, 0], in_=bd.rearrange("(c p) -> p c", p=128))
    nc.scalar.dma_start(out=q_nat[:, 8:12, :], in_=q_view[:, 8:12, :])
    nc.sync.dma_start(out=q_nat[:, 12:16, :], in_=q_view[:, 12:16, :])
    c_load(1)
    c_load(2)
    c_load(3)

    # --- C^T: ct[h, dm, hc, d] = C^{(pa)}[dc*512, dm, hc, d] ---
    cts = {}

    def c_prep(dc, on_pe=False):
        cts[dc] = ctpool.tile([128, NDM, HC, 128], LS, tag="ct", name=f"ct{dc}")
        if on_pe:
            for hc in range(HC):
                pt = psum_t.tile([128, 512], F32, tag="pt", name=f"ptc{dc}_{hc}")
                for dm in range(NDM):
                    nc.tensor.transpose(
                        pt[:, dm * 128:(dm + 1) * 128],
                        c_nats[dc][:, dm, hc * 128:(hc + 1) * 128], ident32[:])
                nc.vector.tensor_copy(
                    cts[dc][:, :, hc, :], pt.rearrange("p (a b) -> p a b", a=4))
        else:
            nc.sync.dma_start_transpose(cts[dc][:], c_bfs[dc][:])

    # qt[h, g, k, hc, q] = Q[g*512 + k*128 + q, hc*128 + h]
    # All transposes straight from f32 on the PE (2 cyc/row); the
    # psum->SBUF copy casts to fp16, so no separate cast stage exists on
    # any startup chain.
    qt = qstat.tile([128, NQG, 4, HC, 128], LS)

    def qt_pe(g):
        for hc in range(HC):
            pt = psum_t.tile([128, 512], F32, tag="pt", name=f"ptq{g}_{hc}")
            for k in range(4):
                nc.tensor.transpose(
                    pt[:, k * 128:(k + 1) * 128],
                    q_nat[:, g * 4 + k, hc * 128:(hc + 1) * 128], ident32[:])
            nc.vector.tensor_copy(
                qt[:, g, :, hc, :], pt.rearrange("p (a b) -> p a b", a=4))

    qt_pe(0)

    # --- W^T on PE: wt[h, hc, om*128+o] = W[om*128+o, hc*128+h] ---
    wt = qstat.tile([128, HC, H], LS)
    for om in range(HC):
        for hc in range(HC):
            pt = psum_t.tile([128, 128], F32, tag="pt", name=f"ptw{om}_{hc}")
            nc.tensor.transpose(pt[:], w_nat[:, om, hc * 128:(hc + 1) * 128],
                                ident32[:])
            nc.vector.tensor_copy(wt[:, hc, om * 128:(om + 1) * 128], pt[:])

    qt_pe(1)
    qt_pe(2)
    qt_pe(3)

    c_prep(0, on_pe=True)

    # --- per-group pipeline: linear+gelu -> QpT -> Qp ---
    qpt = qstat.tile([128, HC, QL], LS)
    # Row pitch padded to 264 cols (528B, 16B-aligned row starts); only
    # cols 0..256 are written/read.
    qp = qstat.tile([128, NQT, 264], BF16)

    def lin_part(qg):
        # linear + gelu for this 512-wide q block
        for om in range(HC):
            pl = psum_l.tile([128, 512], F32, tag="pl", name=f"plin{qg}_{om}")
            for hc in range(HC):
                nc.tensor.matmul(
                    pl[:],
                    wt[:, hc, om * 128:(om + 1) * 128],
                    qt[:, qg, :, hc, :],
                    start=(hc == 0),
                    stop=(hc == HC - 1),
                )
            nc.scalar.activation(
                qpt[:, om, qg * 512:(qg + 1) * 512], pl[:], AF.Gelu,
                bias=bias[:, om, :], scale=1.0,
            )

    def qpb_part(qg):
        # Qp natural for this group (PE back-transpose)
        for om in range(HC):
            pt = psum_t.tile([128, 512], LS, tag="pt", name=f"ptp{qg}_{om}")
            for k in range(4):
                qi = qg * 4 + k
                nc.tensor.transpose(
                    pt[:, k * 128:(k + 1) * 128],
                    qpt[:, om, qi * 128:(qi + 1) * 128], ident[:])
            nc.vector.tensor_copy(
                qp[:, qg * 4:(qg + 1) * 4, om * 128:(om + 1) * 128],
                pt.rearrange("p (a b) -> p a b", a=4))

    lin_part(0)
    qpb_part(0)
    c_prep(1)
    lin_part(1)
    qpb_part(1)
    lin_part(2)
    qpb_part(2)
    lin_part(3)
    qpb_part(3)
    nc.vector.memset(qp[:, :, H:H + 1], 1.0)

    # Lag the attended matmuls behind logits+exp so the PE never waits on
    # the ACT exp latency. Chunk 0 uses a deeper lag: its first exp sits
    # behind gelu-g3 plus the exp-table load on ACT, so the logits run
    # ahead through extra PSUM slots instead of head-of-line blocking.
    LAG = 2
    LAG0 = 2
    for dc in range(NDC):
        lag = LAG0 if dc == 0 else LAG
        c_nat = c_nats[dc]
        ct = cts[dc]
        expt = exppool.tile([128, NQT, DC], BF16, tag="expt")
        pa = [psum_a.tile([128, H + 1], F32, tag=f"a{dm}", name=f"pa{dm}")
              for dm in range(NDM)]
        for step in range(NQT + lag):
            if step == 2 and dc >= 1 and dc + 3 < NDC:
                c_load(dc + 3)
            if step == 6 and dc + 2 < NDC and dc + 2 > 1:
                c_prep(dc + 2)
            if step < NQT:
                qi = step
                if qi in (5, 13) and dc > 0:
                    pl = psum_t.tile([128, DC], F32, tag="pt",
                                     name=f"plx{dc}_{qi}")
                else:
                    pl = psum_l.tile([128, DC], F32, tag="pl")
                for hc in range(HC):
                    nc.tensor.matmul(
                        pl[:],
                        qpt[:, hc, qi * 128:(qi + 1) * 128],
                        ct[:, :, hc, :],
                        start=(hc == 0),
                        stop=(hc == HC - 1),
                    )
                nc.scalar.activation(expt[:, qi, :], pl[:], AF.Exp)
            if step >= lag:
                qj = step - lag
                for dm in range(NDM):
                    nc.tensor.matmul(
                        pa[dm][:],
                        expt[:, qj, dm * 128:(dm + 1) * 128],
                        qp[:, qj, 0:H + 1],
                        start=(qj == 0),
                        stop=(qj == NQT - 1),
                    )

        o_sb = outpool.tile([128, NDM, H], F32)
        if dc == NDC - 1:
            halves = ((0, 1), (1, 2), (2, 3), (3, 4))
        else:
            halves = ((0, 4),)
        for lo, hi in halves:
            for dm in range(lo, hi):
                rec = small.tile([128, 1], F32)
                nc.vector.reciprocal(rec[:], pa[dm][:, H:H + 1])
                nc.vector.scalar_tensor_tensor(
                    o_sb[:, dm, :], pa[dm][:, 0:H], rec[:], c_nat[:, dm, :],
                    ALU.mult, ALU.add,
                )
            nc.sync.dma_start(
                out=Od[dc * DC:(dc + 1) * DC, :]
                .rearrange("(p a) h -> p a h", p=128)[:, lo:hi, :],
                in_=o_sb[:, lo:hi, :])
        del c_nats[dc], cts[dc]
        c_bfs.pop(dc, None)


def build_nc():
    nc = bacc.Bacc("TRN2", target_bir_lowering=False, debug=False,
                   num_devices=N_CORES)
    Qd = nc.dram_tensor("Q", [QL, H], F32, kind="ExternalInput")
    Cd = nc.dram_tensor("C", [D, H], F32, kind="ExternalInput")
    Wd = nc.dram_tensor("W", [H, H], F32, kind="ExternalInput")
    bd = nc.dram_tensor("b", [H], F32, kind="ExternalInput")
    Od = nc.dram_tensor("out", [D, H], F32, kind="ExternalOutput")
    with tile.TileContext(nc) as tc:
        with ExitStack() as ctx:
            build_body(ctx, tc, nc, Qd[:], Cd[:], Wd[:], bd[:], Od[:])
    nc.finalize()
    return nc


_NC = None


def get_nc():
    global _NC
    if _NC is None:
        _NC = build_nc()
    return _NC


def kernel(Q, C, W, b):
    assert Q.shape == (B, QL, H) and C.shape == (B, D, H)
    nc = get_nc()
    in_maps = [
        {
            "Q": np.ascontiguousarray(Q[i], dtype=np.float32),
            "C": np.ascontiguousarray(C[i], dtype=np.float32),
            "W": np.ascontiguousarray(W, dtype=np.float32),
            "b": np.ascontiguousarray(b, dtype=np.float32),
        }
        for i in range(N_CORES)
    ]
    res = run_bass_kernel_spmd(nc, in_maps, core_ids=list(range(N_CORES)))
    return np.stack([res.results[i]["out"] for i in range(N_CORES)], axis=0)


# revision 36
# speedup vs baseline: 1.0212x; 1.0082x over previous
"""CQAttention Trainium2 kernel.

Math (per batch b, H=256, q=2048, d=8192):
  Qp   = gelu(Q @ W.T + b)                       [q, H]
  S    = C @ Qp.T                                [d, q]
  P    = softmax(S, axis=q)
  out  = P @ Qp + C                              [d, H]

Sharding: data-parallel over batch, one batch per NeuronCore (8 cores).

Per-core pipeline:
  - Q and C load with a contiguous per-partition DRAM mapping "(p a) h"
    (one 4KB descriptor per partition instead of 4-16 strided 1KB ones);
    softmax over q is permutation-invariant and the d-mapping only needs
    to be consistent between ct / c_nat / the output store, so the
    row-scramble is free. Loads are f32 HWDGE DMAs split across the two
    DGE sequencers (ACT idle at startup, SP).
  - Startup transposes (W^T, all Q^T groups, C^T chunk 0) run on the PE
    straight from f32 (2 cyc/row; the psum->SBUF copy casts to fp16, so
    no separate cast stage exists on any startup chain). C^T for chunks
    >= 1 runs on the DMA XBAR (dma_start_transpose, 14ns/16x128 tile)
    from a fp16 copy made by the otherwise-idle Pool engine, keeping the
    PE free for matmuls. ~22 dummy matmuls at the very start hold the PE
    busy through the first DMA waits so the p-state ramp (full clock
    after ~3us continuous execution) completes before real work.
  - QpT = gelu(W Q^T + b) with per-partition bias on ACT; Qp (natural,
    bf16) by PE-transposing QpT back, with a ones column so the softmax
    denominator falls out of the attended matmul's PSUM accumulation.
  - Per 512-row chunk of C: logits^T tiles [q=128, d=512] with fp16
    operands (bf16 fails the 2e-2 gate); exp on ACT straight from PSUM
    to bf16 (no max-subtraction: |logits| < ~70 so fp32 exp is safe);
    attended accumulated over 16 q-tiles into PSUM [d=128, 257] whose
    column 256 is the row-sum; fused epilogue
    out = (attended * 1/rowsum) + C in one DVE op per tile.
  - Chunk pipeline: C loads 3 chunks ahead, XBAR transposes 2 ahead,
    attended lags logits/exp by 2 q-tiles.
"""

from contextlib import ExitStack

import numpy as np

import concourse.mybir as mybir
import concourse.tile as tile
from concourse import bacc
from concourse.bass_utils import run_bass_kernel_spmd
from concourse.masks import make_identity

B, QL, D, H = 8, 2048, 8192, 256
N_CORES = 8
F32 = mybir.dt.float32
BF16 = mybir.dt.bfloat16
F16 = mybir.dt.float16

HC = H // 128      # feature chunks (2)
NQT = QL // 128    # q tiles (16)
NQG = NQT // 4     # q groups of 4 tiles (4)
DC = 512           # d-chunk size
NDC = D // DC      # d chunks (16)
NDM = DC // 128    # d tiles per chunk (4)

LS = F16  # logits-matmul operand dtype

AF = mybir.ActivationFunctionType
ALU = mybir.AluOpType


def build_body(ctx: ExitStack, tc: tile.TileContext, nc, Qd, Cd, Wd, bd, Od):
    singles = ctx.enter_context(tc.tile_pool(name="singles", bufs=1))
    qstat = ctx.enter_context(tc.tile_pool(name="qstat", bufs=1))
    cpool = ctx.enter_context(tc.tile_pool(name="cpool", bufs=5))
    ctpool = ctx.enter_context(tc.tile_pool(name="ctp", bufs=4))
    exppool = ctx.enter_context(tc.tile_pool(name="expp", bufs=2))
    outpool = ctx.enter_context(tc.tile_pool(name="outp", bufs=3))
    small = ctx.enter_context(tc.tile_pool(name="small", bufs=4))
    psum_l = ctx.enter_context(tc.tile_pool(name="psl", bufs=2, space="PSUM"))
    psum_t = ctx.enter_context(tc.tile_pool(name="pst", bufs=2, space="PSUM"))
    psum_a = ctx.enter_context(tc.tile_pool(name="psa", bufs=1, space="PSUM"))

    ident = singles.tile([128, 128], LS)
    make_identity(nc, ident)

    # Dummy matmuls to ramp the PE out of its p-state throttle (full clock
    # needs ~3us of continuous execution) while the first loads are in
    # flight; results are never read.
    warm = singles.tile([128, 512], LS, name="warm")
    nc.gpsimd.memset(warm[:], 0.0)
    warm_o = singles.tile([128, 1], F32, name="warm_o")
    nc.scalar.activation(warm_o[:], warm[:, 0:1], AF.Gelu)
    def warmup(n):
        for _ in range(n):
            pw = psum_l.tile([128, 512], F32, tag="pl", name="warmup")
            nc.tensor.matmul(pw[:], ident[:], warm[:], start=True, stop=True)

    warmup(16)

    # --- loads alternate between the two HWDGE queues (ACT idle at start,
    # SP) so the four Q transfers overlap ---
    ident32 = singles.tile([128, 128], F32)
    make_identity(nc, ident32)

    q_nat = cpool.tile([128, NQT, H], F32, tag="qnat", bufs=1)
    q_view = Qd.rearrange("(p a) h -> p a h", p=128)
    nc.sync.dma_start(out=q_nat[:, 0:4, :], in_=q_view[:, 0:4, :])
    w_nat = singles.tile([128, HC, H], F32)  # [o in-chunk, om, h]
    nc.scalar.dma_start(out=w_nat[:],
                        in_=Wd.rearrange("(a p) h -> p a h", p=128))

    c_nats = {}
    c_bfs = {}

    def c_load(dc, make_bf=True):
        c_nats[dc] = cpool.tile([128, NDM, H], F32, tag="cnat", name=f"cnat{dc}")
        nc.sync.dma_start(
            out=c_nats[dc][:],
            in_=Cd[dc * DC:(dc + 1) * DC, :].rearrange("(p a) h -> p a h", p=128))
        if make_bf:
            c_bfs[dc] = cpool.tile([128, NDM, H], LS, tag="cbf", name=f"cbf{dc}")
            nc.gpsimd.tensor_copy(c_bfs[dc][:], c_nats[dc][:])

    c_load(0, make_bf=False)
    nc.sync.dma_start(out=q_nat[:, 4:8, :], in_=q_view[:, 4:8, :])
    bias = singles.tile([128, HC, 1], F32)
    nc.scalar.dma_start(out=bias[:, :, 0], in_=bd.rearrange("(c p) -> p c", p=128))
    nc.scalar.dma_start(out=q_nat[:, 8:12, :], in_=q_view[:, 8:12, :])
    nc.sync.dma_start(out=q_nat[:, 12:16, :], in_=q_view[:, 12:16, :])
    c_load(1)
    c_load(2)
    c_load(3)

    # --- C^T: ct[h, dm, hc, d] = C^{(pa)}[dc*512, dm, hc, d] ---
    cts = {}

    def c_prep(dc, on_pe=False):
        cts[dc] = ctpool.tile([128, NDM, HC, 128], LS, tag="ct", name=f"ct{dc}")
        if on_pe:
            for hc in range(HC):
                pt = psum_t.tile([128, 512], F32, tag="pt", name=f"ptc{dc}_{hc}")
                for dm in range(NDM):
                    nc.tensor.transpose(
                        pt[:, dm * 128:(dm + 1) * 128],
                        c_nats[dc][:, dm, hc * 128:(hc + 1) * 128], ident32[:])
                nc.vector.tensor_copy(
                    cts[dc][:, :, hc, :], pt.rearrange("p (a b) -> p a b", a=4))
        else:
            nc.sync.dma_start_transpose(cts[dc][:], c_bfs[dc][:])

    # qt[h, g, k, hc, q] = Q[g*512 + k*128 + q, hc*128 + h]
    # All transposes straight from f32 on the PE (2 cyc/row); the
    # psum->SBUF copy casts to fp16, so no separate cast stage exists on
    # any startup chain.
    qt = qstat.tile([128, NQG, 4, HC, 128], LS)

    def qt_pe(g):
        for hc in range(HC):
            pt = psum_t.tile([128, 512], F32, tag="pt", name=f"ptq{g}_{hc}")
            for k in range(4):
                nc.tensor.transpose(
                    pt[:, k * 128:(k + 1) * 128],
                    q_nat[:, g * 4 + k, hc * 128:(hc + 1) * 128], ident32[:])
            nc.vector.tensor_copy(
                qt[:, g, :, hc, :], pt.rearrange("p (a b) -> p a b", a=4))

    qt_pe(0)

    # --- W^T on PE: wt[h, hc, om*128+o] = W[om*128+o, hc*128+h] ---
    wt = qstat.tile([128, HC, H], LS)
    for om in range(HC):
        for hc in range(HC):
            pt = psum_t.tile([128, 128], F32, tag="pt", name=f"ptw{om}_{hc}")
            nc.tensor.transpose(pt[:], w_nat[:, om, hc * 128:(hc + 1) * 128],
                                ident32[:])
            nc.vector.tensor_copy(wt[:, hc, om * 128:(om + 1) * 128], pt[:])

    qt_pe(1)
    qt_pe(2)
    qt_pe(3)

    c_prep(0, on_pe=True)

    # --- per-group pipeline: linear+gelu -> QpT -> Qp ---
    qpt = qstat.tile([128, HC, QL], LS)
    # Row pitch padded to 264 cols (528B, 16B-aligned row starts); only
    # cols 0..256 are written/read.
    qp = qstat.tile([128, NQT, 264], BF16)

    def lin_part(qg):
        # linear + gelu for this 512-wide q block
        for om in range(HC):
            pl = psum_l.tile([128, 512], F32, tag="pl", name=f"plin{qg}_{om}")
            for hc in range(HC):
                nc.tensor.matmul(
                    pl[:],
                    wt[:, hc, om * 128:(om + 1) * 128],
                    qt[:, qg, :, hc, :],
                    start=(hc == 0),
                    stop=(hc == HC - 1),
                )
            nc.scalar.activation(
                qpt[:, om, qg * 512:(qg + 1) * 512], pl[:], AF.Gelu,
                bias=bias[:, om, :], scale=1.0,
            )

    def qpb_part(qg):
        # Qp natural for this group (PE back-transpose)
        for om in range(HC):
            pt = psum_t.tile([128, 512], LS, tag="pt", name=f"ptp{qg}_{om}")
            for k in range(4):
                qi = qg * 4 + k
                nc.tensor.transpose(
                    pt[:, k * 128:(k + 1) * 128],
                    qpt[:, om, qi * 128:(qi + 1) * 128], ident[:])
            nc.vector.tensor_copy(
                qp[:, qg * 4:(qg + 1) * 4, om * 128:(om + 1) * 128],
                pt.rearrange("p (a b) -> p a b", a=4))

    lin_part(0)
    qpb_part(0)
    c_prep(1)
    lin_part(1)
    qpb_part(1)
    lin_part(2)
    qpb_part(2)
    lin_part(3)
    qpb_part(3)
    nc.vector.memset(qp[:, :, H:H + 1], 1.0)

    # Lag the attended matmuls behind logits+exp so the PE never waits on
    # the ACT exp latency. Chunk 0 uses a deeper lag: its first exp sits
    # behind gelu-g3 plus the exp-table load on ACT, so the logits run
    # ahead through extra PSUM slots instead of head-of-line blocking.
    LAG = 2
    LAG0 = 2
    for dc in range(NDC):
        lag = LAG0 if dc == 0 else LAG
        c_nat = c_nats[dc]
        ct = cts[dc]
        expt = exppool.tile([128, NQT, DC], BF16, tag="expt")
        pa = [psum_a.tile([128, H + 1], F32, tag=f"a{dm}", name=f"pa{dm}")
              for dm in range(NDM)]
        for step in range(NQT + lag):
            if step == 2 and dc >= 1 and dc + 3 < NDC:
                c_load(dc + 3)
            if step == 6 and dc + 2 < NDC and dc + 2 > 1:
                c_prep(dc + 2)
            if step < NQT:
                qi = step
                if qi in (5, 13) and dc > 0:
                    pl = psum_t.tile([128, DC], F32, tag="pt",
                                     name=f"plx{dc}_{qi}")
                else:
                    pl = psum_l.tile([128, DC], F32, tag="pl")
                for hc in range(HC):
                    nc.tensor.matmul(
                        pl[:],
                        qpt[:, hc, qi * 128:(qi + 1) * 128],
                        ct[:, :, hc, :],
                        start=(hc == 0),
                        stop=(hc == HC - 1),
                    )
                nc.scalar.activation(expt[:, qi, :], pl[:], AF.Exp)
            if step >= lag:
                qj = step - lag
                for dm in range(NDM):
                    nc.tensor.matmul(
                        pa[dm][:],
                        expt[:, qj, dm * 128:(dm + 1) * 128],
                        qp[:, qj, 0:H + 1],
                        start=(qj == 0),
                        stop=(qj == NQT - 1),
                    )

        o_sb = outpool.tile([128, NDM, H], F32)
        if dc == NDC - 1:
            halves = ((0, 1), (1, 2), (2, 3), (3, 4))
        else:
            halves = ((0, 4),)
        for lo, hi in halves:
            for dm in range(lo, hi):
                rec = small.tile([128, 1], F32)
                nc.vector.reciprocal(rec[:], pa[dm][:, H:H + 1])
                nc.vector.scalar_tensor_tensor(
                    o_sb[:, dm, :], pa[dm][:, 0:H], rec[:], c_nat[:, dm, :],
                    ALU.mult, ALU.add,
                )
            nc.sync.dma_start(
                out=Od[dc * DC:(dc + 1) * DC, :]
                .rearrange("(p a) h -> p a h", p=128)[:, lo:hi, :],
                in_=o_sb[:, lo:hi, :])
        del c_nats[dc], cts[dc]
        c_bfs.pop(dc, None)


def build_nc():
    nc = bacc.Bacc("TRN2", target_bir_lowering=False, debug=False,
                   num_devices=N_CORES)
    Qd = nc.dram_tensor("Q", [QL, H], F32, kind="ExternalInput")
    Cd = nc.dram_tensor("C", [D, H], F32, kind="ExternalInput")
    Wd = nc.dram_tensor("W", [H, H], F32, kind="ExternalInput")
    bd = nc.dram_tensor("b", [H], F32, kind="ExternalInput")
    Od = nc.dram_tensor("out", [D, H], F32, kind="ExternalOutput")
    with tile.TileContext(nc) as tc:
        with ExitStack() as ctx:
            build_body(ctx, tc, nc, Qd[:], Cd[:], Wd[:], bd[:], Od[:])
    nc.finalize()
    return nc


_NC = None


def get_nc():
    global _NC
    if _NC is None:
        _NC = build_nc()
    return _NC


def kernel(Q, C, W, b):
    assert Q.shape == (B, QL, H) and C.shape == (B, D, H)
    nc = get_nc()
    in_maps = [
        {
            "Q": np.ascontiguousarray(Q[i], dtype=np.float32),
            "C": np.ascontiguousarray(C[i], dtype=np.float32),
            "W": np.ascontiguousarray(W, dtype=np.float32),
            "b": np.ascontiguousarray(b, dtype=np.float32),
        }
        for i in range(N_CORES)
    ]
    res = run_bass_kernel_spmd(nc, in_maps, core_ids=list(range(N_CORES)))
    return np.stack([res.results[i]["out"] for i in range(N_CORES)], axis=0)
